# revision 3
# baseline (speedup 1.0000x reference)
"""JointLoss Trainium2 kernel.

Math (see reference):
  loss_pos[i] = ||f_i - agents[l_i]||^2
  neg[i]      = mean over masked j of relu(1 - dist[i,j]);  dist = f2 + a2 - 2 f.a
  out         = (sum loss_pos + sum neg_src + sum neg_tgt) / (B + n_valid)

Device strategy (per core, 2048 rows, data-parallel over B):
  One K=65 DoubleRow fp8 matmul per PSUM chunk computes
    pv = 2 f.a - a2 + (1 - f2) = 1 - dist
  directly: 64 partitions x 2 planes carry the 128 feature dims at 0.5
  cycles/column, and partition 65 carries a rank-2 update
  [ones x (-a2_j) + (1 - f2_i) x ones] so PSUM holds the full hinge argument.
  Masked hinge sums per 128-row tile are split across engines (12 "D" tiles +
  20 "A" tiles, chunk-interleaved in D+A pairs so each in-order engine
  stream stays dense):
    D: DVE scalar_tensor_tensor (relu(pv) * m with fused row-sum) from PSUM
    A: ACT relu PSUM->SBUF bf16 -> Pool TT (h *= m) -> DVE 4x tensor_scalar
       (copy with fused row-sum)   [Pool has no TensorScalar opcode on trn2]
  Masks (sim > 0.5, labels excluded) arrive as uint8; per-row counts are
  host-side (GPSIMD cannot reduce the free axis; one fused DVE op can't
  produce two reductions). f2/a2 norms, loss_pos, and the final reduction
  stay on device. Per-core partials [term_sum, n_valid] combine on host.
  Cost-model span: ~115us/core vs ~490us for the fp32 4-DVE-pass baseline.
"""

import numpy as np
import ml_dtypes

B, C, D = 16384, 4000, 128
NCORES = 8
BS = B // NCORES  # 2048 rows per core
NIB = BS // 128  # 16 row blocks per core
NSTREAM = 2
NT = NSTREAM * NIB  # 32 tiles per core
PCHUNKS = [(0, 2048), (2048, 4000)]

FP8 = ml_dtypes.float8_e4m3
BF16 = ml_dtypes.bfloat16

# D-path (DVE-direct) tiles vs A-path (ACT relu + Pool TT + DVE ts-accum)
N_DVE = 12

_CACHE = {}


def _build_nc():
    import concourse.bacc as bacc
    import concourse.tile as tile
    from concourse import mybir

    f32 = mybir.dt.float32
    bf16 = mybir.dt.bfloat16
    u8 = mybir.dt.uint8
    fp8 = mybir.dt.float8e4
    Alu = mybir.AluOpType
    Act = mybir.ActivationFunctionType
    PM = mybir.MatmulPerfMode
    X = mybir.AxisListType.X

    nc = bacc.Bacc(
        "TRN2",
        target_bir_lowering=False,
        debug=False,
        enable_asserts=False,
        num_devices=NCORES,
    )

    fT8_d = nc.dram_tensor("fT8", (65, 2 * BS), fp8, kind="ExternalInput").ap()
    ftT8_d = nc.dram_tensor("ftT8", (65, 2 * BS), fp8, kind="ExternalInput").ap()
    fTb_d = nc.dram_tensor("fTb", (128, BS), bf16, kind="ExternalInput").ap()
    ftTb_d = nc.dram_tensor("ftTb", (128, BS), bf16, kind="ExternalInput").ap()
    alTb_d = nc.dram_tensor("alTb", (128, BS), bf16, kind="ExternalInput").ap()
    sqaT_d = nc.dram_tensor("sqaT", (128, C), bf16, kind="ExternalInput").ap()
    rhs8_d = nc.dram_tensor("rhs8", (65, 2 * C), fp8, kind="ExternalInput").ap()
    msrc_d = nc.dram_tensor("msrc", (BS, C), u8, kind="ExternalInput").ap()
    mtgt_d = nc.dram_tensor("mtgt", (BS, C), u8, kind="ExternalInput").ap()
    cnt_d = nc.dram_tensor("cnt", (128, NT), f32, kind="ExternalInput").ap()
    out_d = nc.dram_tensor("out", (1, 2), f32, kind="ExternalOutput").ap()

    with tile.TileContext(nc) as tc:
        with (
            tc.tile_pool(name="const", bufs=1) as const,
            tc.tile_pool(name="setup", bufs=1) as setup,
            tc.tile_pool(name="mwork", bufs=3) as mwork,
            tc.tile_pool(name="wwork", bufs=2) as wwork,
            tc.tile_pool(name="hwork", bufs=2) as hwork,
            tc.tile_pool(name="psum", bufs=2, space="PSUM") as psum,
        ):
            ones_col = const.tile([128, 1], f32)
            nc.vector.memset(ones_col, 1.0)
            ones_col_bf = const.tile([128, 1], bf16)
            nc.vector.memset(ones_col_bf, 1.0)

            # --- persistent operands (row 64: ones/zeros prepacked by host) ---
            # Warm the ACT function table immediately (LoadActFuncSet ~1.3us).
            actwarm = const.tile([1, 1], f32)
            nc.scalar.activation(out=actwarm, in_=ones_col[0:1, 0:1], func=Act.Copy)

            # DMA prefix order gates startup: sqa c0, fTb, sqa c1, rhs8, fT8,
            # mask group 0, then stream-1 uploads.
            sqa = setup.tile([128, C], bf16, tag="sqa")
            nc.sync.dma_start(out=sqa[:, 0:2048], in_=sqaT_d[:, 0:2048])
            xTb = []
            for s in range(2):
                xt = setup.tile([128, BS], bf16, tag=f"xTb{s}")
                xTb.append(xt)
            nc.sync.dma_start(out=xTb[0], in_=fTb_d)
            nc.sync.dma_start(out=sqa[:, 2048:C], in_=sqaT_d[:, 2048:C])
            rhs65 = const.tile([65, 2 * C], fp8)
            nc.sync.dma_start(out=rhs65, in_=rhs8_d)
            lhs65 = []
            for s in range(2):
                lt = const.tile([65, 2 * BS], fp8, tag=f"lhs{s}")
                lhs65.append(lt)
            nc.sync.dma_start(out=lhs65[0], in_=fT8_d)
            cnt_t = const.tile([128, NT], f32)
            sw_st = const.tile([128, 2 * NT], f32)  # 2 chunk-sums per tile

            GRP = 4  # tiles per mask DMA
            m_groups = {}  # group index -> tile

            def get_mask(t):
                s, ib = t // NIB, t % NIB
                g = t // GRP
                if g not in m_groups:
                    msrc = [msrc_d, mtgt_d][s]
                    mg_ap = msrc.rearrange("(q p) c -> p q c", p=128)
                    m_g = mwork.tile([128, GRP * C], u8, tag="m")
                    gib = (ib // GRP) * GRP
                    if g == 0:
                        # tile 0's mask lands first; rest of the group follows
                        nc.sync.dma_start(out=m_g[:, 0:C], in_=mg_ap[:, 0:1, :])
                        nc.sync.dma_start(out=m_g[:, C:], in_=mg_ap[:, 1:GRP, :])
                    else:
                        nc.sync.dma_start(
                            out=m_g, in_=mg_ap[:, gib : gib + GRP, :]
                        )
                    m_groups[g] = m_g
                return m_groups[g][:, (t % GRP) * C : (t % GRP + 1) * C]

            get_mask(0)  # prefetch group 0 ahead of stream-1 uploads
            nc.sync.dma_start(out=xTb[1], in_=ftTb_d)
            nc.sync.dma_start(out=lhs65[1], in_=ftT8_d)

            def emit_bias(s):
                sqf = setup.tile([128, BS], bf16, tag=f"sqf{s}")
                nc.gpsimd.tensor_tensor(out=sqf, in0=xTb[s], in1=xTb[s], op=Alu.mult)
                ps = psum.tile([128, 2048], f32, tag="ps")
                for k in range(0, BS, 512):
                    nc.tensor.matmul(
                        ps[0:1, k : k + 512],
                        lhsT=ones_col_bf,
                        rhs=sqf[:, k : k + 512],
                        start=True,
                        stop=True,
                    )
                nc.scalar.activation(
                    out=lhs65[s][64:65, BS : 2 * BS],
                    in_=ps[0:1, :BS],
                    func=Act.Copy,
                    scale=-1.0,
                    bias=1.0,
                )

            def emit_nega2(js, je):
                ps = psum.tile([128, 2048], f32, tag="ps")
                for k in range(js, je, 512):
                    n = min(512, je - k)
                    nc.tensor.matmul(
                        ps[0:1, k - js : k - js + n],
                        lhsT=ones_col_bf,
                        rhs=sqa[:, k : k + n],
                        start=True,
                        stop=True,
                    )
                nc.scalar.activation(
                    out=rhs65[64:65, js:je],
                    in_=ps[0:1, : je - js],
                    func=Act.Copy,
                    scale=-1.0,
                )

            # interleave so PE/ACT/PSUM never block each other:
            emit_nega2(0, 2048)
            emit_bias(0)
            emit_nega2(2048, C)

            lpcol = const.tile([128, 1], f32)

            def emit_loss_pos():
                # off the critical path: emitted after the main loop
                nc.sync.dma_start(out=cnt_t, in_=cnt_d)
                alTb = setup.tile([128, BS], bf16, tag="alTb")
                nc.sync.dma_start(out=alTb, in_=alTb_d)
                dT = setup.tile([128, BS], bf16, tag="dT")
                nc.gpsimd.tensor_tensor(out=dT, in0=xTb[0], in1=alTb, op=Alu.subtract)
                dsc = setup.tile([128, BS], bf16, tag="dsc")
                nc.vector.scalar_tensor_tensor(
                    out=dsc,
                    in0=dT,
                    scalar=1.0,
                    in1=dT,
                    op0=Alu.mult,
                    op1=Alu.mult,
                    accum_out=lpcol,
                )

            # --- main loop over 32 tiles ---
            lhs_aps = [
                lt.rearrange("k (two m) -> k two m", two=2) for lt in lhs65
            ]
            rhs_ap = rhs65.rearrange("k (two n) -> k two n", two=2)
            # schedule: 13 (D,A) pairs + 6 A-singles, chunk-interleaved so
            # every engine's in-order stream stays dense.
            n_pairs = N_DVE
            n_single = NT - 2 * n_pairs
            slots = []  # list of ("P", tD, tA) or ("S", tA)
            ti = 0
            placed_p = placed_s = 0
            for k in range(n_pairs + n_single):
                if placed_s * n_pairs <= placed_p * n_single - n_single // 2:
                    slots.append(("S", ti))
                    ti += 1
                    placed_s += 1
                else:
                    slots.append(("P", ti, ti + 1))
                    ti += 2
                    placed_p += 1
            # end on a single so ACT/Pool drain alongside DVE's last pair
            for i in range(len(slots) - 1, -1, -1):
                if slots[i][0] == "S":
                    slots.append(slots.pop(i))
                    break

            def mm(t, pv, js, je):
                s, ib = t // NIB, t % NIB
                for k in range(js, je, 512):
                    kn = min(512, je - k)
                    nc.tensor.matmul(
                        pv[:, k - js : k - js + kn],
                        lhsT=lhs_aps[s][:, :, ib * 128 : (ib + 1) * 128],
                        rhs=rhs_ap[:, :, k : k + kn],
                        start=True,
                        stop=True,
                        perf_mode=PM.DoubleRow,
                    )

            for si, slot in enumerate(slots):
                if si == 2:
                    emit_bias(1)
                if slot[0] == "P":
                    _, tD, tA = slot
                    mD, mA = get_mask(tD), get_mask(tA)
                    wD = wwork.tile([128, C], bf16, tag="w")
                    hA = hwork.tile([128, C], bf16, tag="h")
                    pvs = {}
                    for ci, (js, je) in enumerate(PCHUNKS):
                        n = je - js
                        pvD = psum.tile([128, 2048], f32, tag="ps")
                        mm(tD, pvD, js, je)
                        pvA = psum.tile([128, 2048], f32, tag="ps")
                        mm(tA, pvA, js, je)
                        nc.vector.scalar_tensor_tensor(
                            out=wD[:, js:je],
                            in0=pvD[:, :n],
                            scalar=0.0,
                            in1=mD[:, js:je],
                            op0=Alu.max,
                            op1=Alu.mult,
                            accum_out=sw_st[:, 2 * tD + ci : 2 * tD + ci + 1],
                        )
                        nc.scalar.activation(
                            out=hA[:, js:je], in_=pvA[:, :n], func=Act.Relu
                        )
                    for ci, (js, je) in enumerate(PCHUNKS):
                        nc.gpsimd.tensor_tensor(
                            out=hA[:, js:je],
                            in0=hA[:, js:je],
                            in1=mA[:, js:je],
                            op=Alu.mult,
                        )
                        nc.vector.tensor_scalar(
                            hA[:, js:je],
                            hA[:, js:je],
                            1.0,
                            None,
                            Alu.mult,
                            Alu.add,
                            accum_out=sw_st[:, 2 * tA + ci : 2 * tA + ci + 1],
                        )
                else:
                    _, tA = slot
                    mA = get_mask(tA)
                    hA = hwork.tile([128, C], bf16, tag="h")
                    for ci, (js, je) in enumerate(PCHUNKS):
                        n = je - js
                        pvA = psum.tile([128, 2048], f32, tag="ps")
                        mm(tA, pvA, js, je)
                        nc.scalar.activation(
                            out=hA[:, js:je], in_=pvA[:, :n], func=Act.Relu
                        )
                        nc.gpsimd.tensor_tensor(
                            out=hA[:, js:je],
                            in0=hA[:, js:je],
                            in1=mA[:, js:je],
                            op=Alu.mult,
                        )
                        nc.vector.tensor_scalar(
                            hA[:, js:je],
                            hA[:, js:je],
                            1.0,
                            None,
                            Alu.mult,
                            Alu.add,
                            accum_out=sw_st[:, 2 * tA + ci : 2 * tA + ci + 1],
                        )

            emit_loss_pos()

            # --- finalize ---
            with tc.tile_pool(name="fin", bufs=1) as fin:
                swp = sw_st.rearrange("p (t c) -> p t c", c=2)
                swt = fin.tile([128, NT], f32)
                nc.vector.tensor_tensor(
                    out=swt, in0=swp[:, :, 0], in1=swp[:, :, 1], op=Alu.add
                )
                den = fin.tile([128, NT], f32)
                nc.vector.tensor_scalar(den, cnt_t, 1.0, None, Alu.max)
                rec = fin.tile([128, NT], f32)
                nc.vector.reciprocal(rec, den)
                neg = fin.tile([128, NT], f32)
                nc.vector.tensor_tensor(out=neg, in0=swt, in1=rec, op=Alu.mult)
                valid = fin.tile([128, NT], f32)
                nc.vector.tensor_scalar(valid, cnt_t, 0.0, None, Alu.is_gt)
                pack = fin.tile([128, 2], f32)
                nc.vector.tensor_reduce(pack[:, 0:1], neg, axis=X, op=Alu.add)
                nc.vector.tensor_reduce(pack[:, 1:2], valid, axis=X, op=Alu.add)
                psf = psum.tile([128, 2048], f32, tag="ps")
                nc.tensor.matmul(
                    psf[0:1, 0:2], lhsT=ones_col, rhs=pack, start=True, stop=True
                )
                nc.tensor.matmul(
                    psf[0:1, 2:3], lhsT=lpcol, rhs=ones_col, start=True, stop=True
                )
                outt = fin.tile([1, 3], f32)
                nc.scalar.activation(out=outt, in_=psf[0:1, 0:3], func=Act.Copy)
                outf = fin.tile([1, 2], f32)
                nc.vector.tensor_tensor(
                    out=outf[0:1, 0:1], in0=outt[0:1, 0:1], in1=outt[0:1, 2:3], op=Alu.add
                )
                nc.vector.tensor_copy(outf[0:1, 1:2], outt[0:1, 1:2])
                nc.sync.dma_start(out=out_d, in_=outf)

    nc.compile()
    return nc


def _get_nc():
    if "nc" not in _CACHE:
        _CACHE["nc"] = _build_nc()
    return _CACHE["nc"]


def make_in_maps(features, agents, labels, similarity, features_target, similarity_target):
    labels = np.asarray(labels).astype(np.int64)
    features = np.asarray(features, dtype=np.float32)
    agents = np.asarray(agents, dtype=np.float32)
    features_target = np.asarray(features_target, dtype=np.float32)
    similarity = np.asarray(similarity)
    similarity_target = np.asarray(similarity_target)

    al_full = agents[labels]  # (B, D) f32
    aT2 = (2.0 * agents.T).astype(FP8)  # (D, C)
    rhs8 = np.zeros((65, 2 * C), FP8)
    rhs8[:64] = np.concatenate([aT2[:64], aT2[64:]], axis=1)
    rhs8[64, C:] = FP8(1.0)  # plane1 ones; plane0 gets -a2 on device
    sqaT = np.ascontiguousarray((agents.T.astype(BF16) * agents.T.astype(BF16)))

    cols = np.arange(C, dtype=labels.dtype)[None, :]

    in_maps = []
    for c in range(NCORES):
        r = slice(c * BS, (c + 1) * BS)
        f = features[r]
        ft = features_target[r]
        al = al_full[r]
        lbl = labels[r]

        fT = np.ascontiguousarray(f.T)  # (D, BS) f32
        ftT = np.ascontiguousarray(ft.T)
        fT8 = np.zeros((65, 2 * BS), FP8)
        fT8[:64] = np.concatenate([fT[:64], fT[64:]], axis=1).astype(FP8)
        fT8[64, :BS] = FP8(1.0)  # plane0 ones; plane1 gets bias on device
        ftT8 = np.zeros((65, 2 * BS), FP8)
        ftT8[:64] = np.concatenate([ftT[:64], ftT[64:]], axis=1).astype(FP8)
        ftT8[64, :BS] = FP8(1.0)

        msrc = (similarity[r] > 0.5) & (cols != lbl[:, None])
        mtgt = similarity_target[r] > 0.5
        cnt = np.empty((128, NT), np.float32)
        cnt[:, :NIB] = msrc.sum(axis=1, dtype=np.int32).reshape(NIB, 128).T
        cnt[:, NIB:] = mtgt.sum(axis=1, dtype=np.int32).reshape(NIB, 128).T

        in_maps.append(
            {
                "fT8": fT8,
                "ftT8": ftT8,
                "fTb": np.ascontiguousarray(fT.astype(BF16)),
                "ftTb": np.ascontiguousarray(ftT.astype(BF16)),
                "alTb": np.ascontiguousarray(al.T.astype(BF16)),
                "sqaT": sqaT,
                "rhs8": rhs8,
                "msrc": np.ascontiguousarray(msrc.view(np.uint8)),
                "mtgt": np.ascontiguousarray(mtgt.view(np.uint8)),
                "cnt": cnt,
            }
        )
    return in_maps


def kernel(features, agents, labels, similarity, features_target, similarity_target):
    from concourse import bass_utils

    nc = _get_nc()
    in_maps = make_in_maps(
        features, agents, labels, similarity, features_target, similarity_target
    )
    res = bass_utils.run_bass_kernel_spmd(
        nc, in_maps, core_ids=list(range(NCORES)), trace=False
    )
    _CACHE["last_results"] = res
    parts = np.stack([r["out"][0] for r in res.results])  # [8, 2]
    term_sum = float(parts[:, 0].sum())
    n_valid = float(parts[:, 1].sum())
    return np.float32(term_sum / (B + n_valid))


# revision 4
# speedup vs baseline: 1.4090x; 1.4090x over previous
"""JointLoss Trainium2 kernel.

Math (see reference):
  loss_pos[i] = ||f_i - agents[l_i]||^2
  neg[i]      = mean over masked j of relu(1 - dist[i,j]);  dist = f2 + a2 - 2 f.a
  out         = (sum loss_pos + sum neg_src + sum neg_tgt) / (B + n_valid)

Device strategy (per core, 2048 rows, data-parallel over B):
  One K=65 DoubleRow fp8 matmul per PSUM chunk computes
    pv = 2 f.a - a2 + (1 - f2) = 1 - dist
  directly: 64 partitions x 2 planes carry the 128 feature dims at 0.5
  cycles/column, and partition 65 carries a rank-2 update
  [ones x (-a2_j) + (1 - f2_i) x ones] so PSUM holds the full hinge argument.
  Masked hinge sums per 128-row tile are split across engines (12 "D" tiles +
  20 "A" tiles, chunk-interleaved in D+A pairs so each in-order engine
  stream stays dense):
    D: DVE scalar_tensor_tensor (relu(pv) * m with fused row-sum) from PSUM
    A: ACT relu PSUM->SBUF bf16 -> Pool TT (h *= m) -> DVE 4x tensor_scalar
       (copy with fused row-sum)   [Pool has no TensorScalar opcode on trn2]
  Masks (sim > 0.5, labels excluded) arrive as uint8; per-row counts are
  host-side (GPSIMD cannot reduce the free axis; one fused DVE op can't
  produce two reductions). f2/a2 norms, loss_pos, and the final reduction
  stay on device. Per-core partials [term_sum, n_valid] combine on host.
  Cost-model span: ~114us/core vs ~490us for the fp32 4-DVE-pass baseline.
"""

import numpy as np
import ml_dtypes

B, C, D = 16384, 4000, 128
NCORES = 8
BS = B // NCORES  # 2048 rows per core
NIB = BS // 128  # 16 row blocks per core
NSTREAM = 2
NT = NSTREAM * NIB  # 32 tiles per core
PCHUNKS = [(0, 2048), (2048, 4000)]

FP8 = ml_dtypes.float8_e4m3
BF16 = ml_dtypes.bfloat16

# D-path (DVE-direct) tiles vs A-path (ACT relu + Pool TT + DVE ts-accum)
N_DVE = 12

_CACHE = {}


def _build_nc():
    import concourse.bacc as bacc
    import concourse.tile as tile
    from concourse import mybir

    f32 = mybir.dt.float32
    bf16 = mybir.dt.bfloat16
    u8 = mybir.dt.uint8
    fp8 = mybir.dt.float8e4
    Alu = mybir.AluOpType
    Act = mybir.ActivationFunctionType
    PM = mybir.MatmulPerfMode
    X = mybir.AxisListType.X

    nc = bacc.Bacc(
        "TRN2",
        target_bir_lowering=False,
        debug=False,
        enable_asserts=False,
        num_devices=NCORES,
    )

    fT8_d = nc.dram_tensor("fT8", (65, 2 * BS), fp8, kind="ExternalInput").ap()
    ftT8_d = nc.dram_tensor("ftT8", (65, 2 * BS), fp8, kind="ExternalInput").ap()
    fTb_d = nc.dram_tensor("fTb", (128, BS), bf16, kind="ExternalInput").ap()
    ftTb_d = nc.dram_tensor("ftTb", (128, BS), bf16, kind="ExternalInput").ap()
    alTb_d = nc.dram_tensor("alTb", (128, BS), bf16, kind="ExternalInput").ap()
    sqaT_d = nc.dram_tensor("sqaT", (128, C), bf16, kind="ExternalInput").ap()
    rhs8_d = nc.dram_tensor("rhs8", (65, 2 * C), fp8, kind="ExternalInput").ap()
    msrc_d = nc.dram_tensor("msrc", (BS, C), u8, kind="ExternalInput").ap()
    mtgt_d = nc.dram_tensor("mtgt", (BS, C), u8, kind="ExternalInput").ap()
    cnt_d = nc.dram_tensor("cnt", (128, NT), f32, kind="ExternalInput").ap()
    out_d = nc.dram_tensor("out", (1, 2), f32, kind="ExternalOutput").ap()

    with tile.TileContext(nc) as tc:
        with (
            tc.tile_pool(name="const", bufs=1) as const,
            tc.tile_pool(name="setup", bufs=1) as setup,
            tc.tile_pool(name="mwork", bufs=3) as mwork,
            tc.tile_pool(name="wwork", bufs=2) as wwork,
            tc.tile_pool(name="hwork", bufs=2) as hwork,
            tc.tile_pool(name="psum", bufs=2, space="PSUM") as psum,
        ):
            ones_col = const.tile([128, 1], f32)
            nc.vector.memset(ones_col, 1.0)
            ones_col_bf = const.tile([128, 1], bf16)
            nc.vector.memset(ones_col_bf, 1.0)

            # --- persistent operands (row 64: ones/zeros prepacked by host) ---
            # Warm the ACT function table immediately (LoadActFuncSet ~1.3us).
            actwarm = const.tile([1, 1], f32)
            nc.scalar.activation(out=actwarm, in_=ones_col[0:1, 0:1], func=Act.Copy)

            # DMA prefix order gates startup: sqa c0, fTb, sqa c1, rhs8, fT8,
            # mask group 0, then stream-1 uploads.
            sqa = setup.tile([128, C], bf16, tag="sqa")
            nc.sync.dma_start(out=sqa[:, 0:2048], in_=sqaT_d[:, 0:2048])
            xTb = []
            for s in range(2):
                xt = setup.tile([128, BS], bf16, tag=f"xTb{s}")
                xTb.append(xt)
            nc.sync.dma_start(out=xTb[0], in_=fTb_d)
            nc.sync.dma_start(out=sqa[:, 2048:C], in_=sqaT_d[:, 2048:C])
            rhs65 = const.tile([65, 2 * C], fp8)
            nc.sync.dma_start(out=rhs65, in_=rhs8_d)
            lhs65 = []
            for s in range(2):
                lt = const.tile([65, 2 * BS], fp8, tag=f"lhs{s}")
                lhs65.append(lt)
            nc.sync.dma_start(out=lhs65[0], in_=fT8_d)
            cnt_t = const.tile([128, NT], f32)
            sw_st = const.tile([128, 2 * NT], f32)  # 2 chunk-sums per tile

            GRP = 4  # tiles per mask DMA
            m_groups = {}  # group index -> tile

            def get_mask(t):
                s, ib = t // NIB, t % NIB
                g = t // GRP
                if g not in m_groups:
                    msrc = [msrc_d, mtgt_d][s]
                    mg_ap = msrc.rearrange("(q p) c -> p q c", p=128)
                    m_g = mwork.tile([128, GRP * C], u8, tag="m")
                    gib = (ib // GRP) * GRP
                    if g == 0:
                        # tile 0's mask lands first; rest of the group follows
                        nc.sync.dma_start(out=m_g[:, 0:C], in_=mg_ap[:, 0:1, :])
                        nc.sync.dma_start(out=m_g[:, C:], in_=mg_ap[:, 1:GRP, :])
                    else:
                        nc.sync.dma_start(
                            out=m_g, in_=mg_ap[:, gib : gib + GRP, :]
                        )
                    m_groups[g] = m_g
                return m_groups[g][:, (t % GRP) * C : (t % GRP + 1) * C]

            get_mask(0)  # prefetch groups 0-1 ahead of stream-1 uploads
            get_mask(4)
            nc.sync.dma_start(out=xTb[1], in_=ftTb_d)
            nc.sync.dma_start(out=lhs65[1], in_=ftT8_d)

            def emit_bias(s):
                sqf = setup.tile([128, BS], bf16, tag=f"sqf{s}")
                nc.gpsimd.tensor_tensor(out=sqf, in0=xTb[s], in1=xTb[s], op=Alu.mult)
                ps = psum.tile([128, 2048], f32, tag="ps")
                for k in range(0, BS, 512):
                    nc.tensor.matmul(
                        ps[0:1, k : k + 512],
                        lhsT=ones_col_bf,
                        rhs=sqf[:, k : k + 512],
                        start=True,
                        stop=True,
                    )
                nc.scalar.activation(
                    out=lhs65[s][64:65, BS : 2 * BS],
                    in_=ps[0:1, :BS],
                    func=Act.Copy,
                    scale=-1.0,
                    bias=1.0,
                )

            def emit_nega2(js, je):
                ps = psum.tile([128, 2048], f32, tag="ps")
                for k in range(js, je, 512):
                    n = min(512, je - k)
                    nc.tensor.matmul(
                        ps[0:1, k - js : k - js + n],
                        lhsT=ones_col_bf,
                        rhs=sqa[:, k : k + n],
                        start=True,
                        stop=True,
                    )
                nc.scalar.activation(
                    out=rhs65[64:65, js:je],
                    in_=ps[0:1, : je - js],
                    func=Act.Copy,
                    scale=-1.0,
                )

            # interleave so PE/ACT/PSUM never block each other:
            emit_nega2(0, 2048)
            emit_bias(0)
            emit_nega2(2048, C)

            lpcol = const.tile([128, 1], f32)

            def emit_loss_pos():
                # off the critical path: emitted after the main loop
                nc.sync.dma_start(out=cnt_t, in_=cnt_d)
                alTb = setup.tile([128, BS], bf16, tag="alTb")
                nc.sync.dma_start(out=alTb, in_=alTb_d)
                dT = setup.tile([128, BS], bf16, tag="dT")
                nc.gpsimd.tensor_tensor(out=dT, in0=xTb[0], in1=alTb, op=Alu.subtract)
                dsc = setup.tile([128, BS], bf16, tag="dsc")
                nc.vector.scalar_tensor_tensor(
                    out=dsc,
                    in0=dT,
                    scalar=1.0,
                    in1=dT,
                    op0=Alu.mult,
                    op1=Alu.mult,
                    accum_out=lpcol,
                )

            # --- main loop over 32 tiles ---
            lhs_aps = [
                lt.rearrange("k (two m) -> k two m", two=2) for lt in lhs65
            ]
            rhs_ap = rhs65.rearrange("k (two n) -> k two n", two=2)
            # schedule: 13 (D,A) pairs + 6 A-singles, chunk-interleaved so
            # every engine's in-order stream stays dense.
            n_pairs = N_DVE
            n_single = NT - 2 * n_pairs
            slots = []  # list of ("P", tD, tA) or ("S", tA)
            ti = 0
            placed_p = placed_s = 0
            for k in range(n_pairs + n_single):
                if placed_s * n_pairs <= placed_p * n_single - n_single // 2:
                    slots.append(("S", ti))
                    ti += 1
                    placed_s += 1
                else:
                    slots.append(("P", ti, ti + 1))
                    ti += 2
                    placed_p += 1
            # end on a single so ACT/Pool drain alongside DVE's last pair
            for i in range(len(slots) - 1, -1, -1):
                if slots[i][0] == "S":
                    slots.append(slots.pop(i))
                    break

            def mm(t, pv, js, je):
                s, ib = t // NIB, t % NIB
                for k in range(js, je, 512):
                    kn = min(512, je - k)
                    nc.tensor.matmul(
                        pv[:, k - js : k - js + kn],
                        lhsT=lhs_aps[s][:, :, ib * 128 : (ib + 1) * 128],
                        rhs=rhs_ap[:, :, k : k + kn],
                        start=True,
                        stop=True,
                        perf_mode=PM.DoubleRow,
                    )

            for si, slot in enumerate(slots):
                if si == 5:
                    emit_bias(1)
                if slot[0] == "P":
                    _, tD, tA = slot
                    mD, mA = get_mask(tD), get_mask(tA)
                    wD = wwork.tile([128, C], bf16, tag="w")
                    hA = hwork.tile([128, C], bf16, tag="h")
                    pvs = {}
                    for ci, (js, je) in enumerate(PCHUNKS):
                        n = je - js
                        pvD = psum.tile([128, 2048], f32, tag="ps")
                        mm(tD, pvD, js, je)
                        pvA = psum.tile([128, 2048], f32, tag="ps")
                        mm(tA, pvA, js, je)
                        nc.vector.scalar_tensor_tensor(
                            out=wD[:, js:je],
                            in0=pvD[:, :n],
                            scalar=0.0,
                            in1=mD[:, js:je],
                            op0=Alu.max,
                            op1=Alu.mult,
                            accum_out=sw_st[:, 2 * tD + ci : 2 * tD + ci + 1],
                        )
                        nc.scalar.activation(
                            out=hA[:, js:je], in_=pvA[:, :n], func=Act.Relu
                        )
                    for ci, (js, je) in enumerate(PCHUNKS):
                        nc.gpsimd.tensor_tensor(
                            out=hA[:, js:je],
                            in0=hA[:, js:je],
                            in1=mA[:, js:je],
                            op=Alu.mult,
                        )
                        nc.vector.tensor_scalar(
                            hA[:, js:je],
                            hA[:, js:je],
                            1.0,
                            None,
                            Alu.mult,
                            Alu.add,
                            accum_out=sw_st[:, 2 * tA + ci : 2 * tA + ci + 1],
                        )
                else:
                    _, tA = slot
                    mA = get_mask(tA)
                    hA = hwork.tile([128, C], bf16, tag="h")
                    for ci, (js, je) in enumerate(PCHUNKS):
                        n = je - js
                        pvA = psum.tile([128, 2048], f32, tag="ps")
                        mm(tA, pvA, js, je)
                        nc.scalar.activation(
                            out=hA[:, js:je], in_=pvA[:, :n], func=Act.Relu
                        )
                        nc.gpsimd.tensor_tensor(
                            out=hA[:, js:je],
                            in0=hA[:, js:je],
                            in1=mA[:, js:je],
                            op=Alu.mult,
                        )
                        nc.vector.tensor_scalar(
                            hA[:, js:je],
                            hA[:, js:je],
                            1.0,
                            None,
                            Alu.mult,
                            Alu.add,
                            accum_out=sw_st[:, 2 * tA + ci : 2 * tA + ci + 1],
                        )

            emit_loss_pos()

            # --- finalize ---
            with tc.tile_pool(name="fin", bufs=1) as fin:
                swp = sw_st.rearrange("p (t c) -> p t c", c=2)
                swt = fin.tile([128, NT], f32)
                nc.vector.tensor_tensor(
                    out=swt, in0=swp[:, :, 0], in1=swp[:, :, 1], op=Alu.add
                )
                den = fin.tile([128, NT], f32)
                nc.vector.tensor_scalar(den, cnt_t, 1.0, None, Alu.max)
                rec = fin.tile([128, NT], f32)
                nc.vector.reciprocal(rec, den)
                neg = fin.tile([128, NT], f32)
                nc.vector.tensor_tensor(out=neg, in0=swt, in1=rec, op=Alu.mult)
                valid = fin.tile([128, NT], f32)
                nc.vector.tensor_scalar(valid, cnt_t, 0.0, None, Alu.is_gt)
                pack = fin.tile([128, 2], f32)
                nc.vector.tensor_reduce(pack[:, 0:1], neg, axis=X, op=Alu.add)
                nc.vector.tensor_reduce(pack[:, 1:2], valid, axis=X, op=Alu.add)
                psf = psum.tile([128, 2048], f32, tag="ps")
                nc.tensor.matmul(
                    psf[0:1, 0:2], lhsT=ones_col, rhs=pack, start=True, stop=True
                )
                nc.tensor.matmul(
                    psf[0:1, 2:3], lhsT=lpcol, rhs=ones_col, start=True, stop=True
                )
                outt = fin.tile([1, 3], f32)
                nc.scalar.activation(out=outt, in_=psf[0:1, 0:3], func=Act.Copy)
                outf = fin.tile([1, 2], f32)
                nc.vector.tensor_tensor(
                    out=outf[0:1, 0:1], in0=outt[0:1, 0:1], in1=outt[0:1, 2:3], op=Alu.add
                )
                nc.vector.tensor_copy(outf[0:1, 1:2], outt[0:1, 1:2])
                nc.sync.dma_start(out=out_d, in_=outf)

    nc.compile()
    return nc


def _get_nc():
    if "nc" not in _CACHE:
        _CACHE["nc"] = _build_nc()
    return _CACHE["nc"]


def make_in_maps(features, agents, labels, similarity, features_target, similarity_target):
    labels = np.asarray(labels).astype(np.int64)
    features = np.asarray(features, dtype=np.float32)
    agents = np.asarray(agents, dtype=np.float32)
    features_target = np.asarray(features_target, dtype=np.float32)
    similarity = np.asarray(similarity)
    similarity_target = np.asarray(similarity_target)

    al_full = agents[labels]  # (B, D) f32
    aT2 = (2.0 * agents.T).astype(FP8)  # (D, C)
    rhs8 = np.zeros((65, 2 * C), FP8)
    rhs8[:64] = np.concatenate([aT2[:64], aT2[64:]], axis=1)
    rhs8[64, C:] = FP8(1.0)  # plane1 ones; plane0 gets -a2 on device
    sqaT = np.ascontiguousarray((agents.T.astype(BF16) * agents.T.astype(BF16)))

    cols = np.arange(C, dtype=labels.dtype)[None, :]

    in_maps = []
    for c in range(NCORES):
        r = slice(c * BS, (c + 1) * BS)
        f = features[r]
        ft = features_target[r]
        al = al_full[r]
        lbl = labels[r]

        fT = np.ascontiguousarray(f.T)  # (D, BS) f32
        ftT = np.ascontiguousarray(ft.T)
        fT8 = np.zeros((65, 2 * BS), FP8)
        fT8[:64] = np.concatenate([fT[:64], fT[64:]], axis=1).astype(FP8)
        fT8[64, :BS] = FP8(1.0)  # plane0 ones; plane1 gets bias on device
        ftT8 = np.zeros((65, 2 * BS), FP8)
        ftT8[:64] = np.concatenate([ftT[:64], ftT[64:]], axis=1).astype(FP8)
        ftT8[64, :BS] = FP8(1.0)

        msrc = (similarity[r] > 0.5) & (cols != lbl[:, None])
        mtgt = similarity_target[r] > 0.5
        cnt = np.empty((128, NT), np.float32)
        cnt[:, :NIB] = msrc.sum(axis=1, dtype=np.int32).reshape(NIB, 128).T
        cnt[:, NIB:] = mtgt.sum(axis=1, dtype=np.int32).reshape(NIB, 128).T

        in_maps.append(
            {
                "fT8": fT8,
                "ftT8": ftT8,
                "fTb": np.ascontiguousarray(fT.astype(BF16)),
                "ftTb": np.ascontiguousarray(ftT.astype(BF16)),
                "alTb": np.ascontiguousarray(al.T.astype(BF16)),
                "sqaT": sqaT,
                "rhs8": rhs8,
                "msrc": np.ascontiguousarray(msrc.view(np.uint8)),
                "mtgt": np.ascontiguousarray(mtgt.view(np.uint8)),
                "cnt": cnt,
            }
        )
    return in_maps


def kernel(features, agents, labels, similarity, features_target, similarity_target):
    from concourse import bass_utils

    nc = _get_nc()
    in_maps = make_in_maps(
        features, agents, labels, similarity, features_target, similarity_target
    )
    res = bass_utils.run_bass_kernel_spmd(
        nc, in_maps, core_ids=list(range(NCORES)), trace=False
    )
    _CACHE["last_results"] = res
    parts = np.stack([r["out"][0] for r in res.results])  # [8, 2]
    term_sum = float(parts[:, 0].sum())
    n_valid = float(parts[:, 1].sum())
    return np.float32(term_sum / (B + n_valid))
